# revision 29
# baseline (speedup 1.0000x reference)
"""Trainium2 Bass kernel for nn_Attention4D (EfficientViT-style attention).

Strategy (8 NeuronCores, data-parallel over batch B=8, one batch element per core):
  - BN folded into conv weights on host.
  - Talking-head-1 folded into per-head-scaled queries (Q2), so q@k contracts
    256 channels at full PE efficiency and th1 disappears.
  - Attention tensor layout: partition dim = (head, 16 queries) interleaved
    (49 groups of 128 partitions), free dim = keys m (784).  Softmax is then
    per-partition (ACT exp with fused accumulate for the denominator; bth1
    rides the free per-partition ACT bias), and talking-head-2 is one
    block-diagonal 128x128 PE matmul per group with the softmax normalization
    folded into the block-diag weights.
  - Relative-position biases are a precomputed fp8 table accumulated into the
    logits PSUM with an identity matmul.
  - attn@v needs keys on partitions, so A2 makes one DRAM round trip through
    dma_start_transpose (two scratch tensors so the read-back overlaps the
    tail of the softmax phase).  V^T is produced directly by the projection.
  - The 3x3 depthwise conv (v_local) runs on the PE as 9 diagonal-weight
    matmuls accumulated into the same PSUM banks as attn@v output.
"""

import sys

sys.path.insert(0, "/opt/trn_rl_repo")

import numpy as np
import ml_dtypes

import concourse.bass as bass
import concourse.tile as tile
from concourse import bacc, mybir
from concourse.bass_utils import run_bass_kernel_spmd

F32 = mybir.dt.float32
BF16 = mybir.dt.bfloat16
FP8 = mybir.dt.float8e4
AF = mybir.ActivationFunctionType
BF = ml_dtypes.bfloat16
F8 = ml_dtypes.float8_e4m3

HEADS, KD, AR, RES, DIM = 8, 32, 4, 28, 384
D = AR * KD            # 128
DH = HEADS * D         # 1024
NH_KD = HEADS * KD     # 256
N = RES * RES          # 784
NG = N // 16           # 49 groups of 16 queries
NGA = 28               # groups in first scratch tensor (output chunk 0)
NGB = 16               # second tensor (groups 28..43)
NGC = NG - NGA - NGB   # third tensor (groups 44..48)
B = 8

_CACHE = {}
LAST_RESULTS = None  # test.py reads exec_time from here


def _build_program():
    nc = bacc.Bacc("TRN2", target_bir_lowering=False, debug=False,
                   enable_asserts=True)

    def din(name, shape, dt=F32):
        return nc.dram_tensor(name, shape, dt, kind="ExternalInput")

    x_c = din("x_c", [128, 3 * N], BF16)
    wq3 = din("wq3", [128, 3 * NH_KD], BF16)
    wk3 = din("wk3", [128, 3 * NH_KD], BF16)
    wv3 = din("wv3", [128, 3 * DH], BF16)
    wp8 = din("wp8", [128, 8 * DIM], BF16)
    vecs = din("vecs", [128, 113])
    w2bd = din("w2bd", [128, 128])
    ident = din("ident", [128, 128], BF16)
    identf8 = din("identf8", [128, 128], FP8)
    abt = din("abt", [NG * 128, N], FP8)

    out = nc.dram_tensor("out", [DIM, N], F32, kind="ExternalOutput")
    a2da = nc.dram_tensor("a2da", [NGA * 128, 896], BF16, kind="Internal")
    a2db = nc.dram_tensor("a2db", [NGB * 128, 896], BF16, kind="Internal")
    a2dc = nc.dram_tensor("a2dc", [NGC * 128, 896], BF16, kind="Internal")

    CH0 = slice(0, 392)          # logits free-dim chunks (within 2-bank psum)
    CH1 = slice(392, 784)
    PS0 = slice(0, 392)          # psum [128,1024]: bank0
    PS1 = slice(512, 904)        # bank1

    def psum2view(ps):
        # [128, 2, 392] view of a 2-bank psum tile's used region
        return ps[:].rearrange("p (a c) -> p a c", c=512)[:, :, 0:392]

    with tile.TileContext(nc) as tc:
        with (
            tc.tile_pool(name="consts", bufs=1) as consts,
            tc.tile_pool(name="persist", bufs=1) as persist,
        ):
            # ---- resident weights/constants -------------------------------
            def load_const(name, src_ap, shape, dt=F32):
                t = consts.tile(shape, dt, tag=name, name=name)
                nc.sync.dma_start(t[:], src_ap)
                return t

            wp_w = consts.tile([128, 8 * DIM], BF16, tag="wp_w", name="wp_w")
            wp_t = [wp_w[:, k * DIM:(k + 1) * DIM] for k in range(8)]
            vec_t = consts.tile([128, 113], F32, tag="vec_t", name="vec_t")
            bq_t = [vec_t[:, k:k + 1] for k in range(2)]
            bk_t = [vec_t[:, 2 + k:3 + k] for k in range(2)]
            bv_t = [vec_t[:, 4 + k:5 + k] for k in range(8)]
            bdw_t = [vec_t[:, 12 + k:13 + k] for k in range(8)]
            bp_t = [vec_t[:, 20 + k:21 + k] for k in range(3)]
            bth1_t = vec_t[:, 23:24]
            bth2_t = vec_t[:, 24:25]
            sq_t = [vec_t[:, 25 + k * 8:33 + k * 8] for k in range(2)]
            wtap_t = [vec_t[:, 41 + g * 9:50 + g * 9] for g in range(8)]
            w2bd_t = consts.tile([128, 128], F32, tag="w2bd", name="w2bd")
            ident_t = consts.tile([128, 128], BF16, tag="ident", name="ident")
            identf8_t = consts.tile([128, 128], FP8, tag="identf8",
                                    name="identf8")

            def load_small_consts():
                nc.sync.dma_start(vec_t[:], vecs.ap()[:])
                nc.sync.dma_start(w2bd_t[:], w2bd.ap()[:])
                nc.sync.dma_start(ident_t[:], ident.ap()[:])
                nc.sync.dma_start(identf8_t[:], identf8.ap()[:])

            # ---- persistent activations -----------------------------------
            vpad = [persist.tile([128, 900], BF16, tag=f"vpad{p}",
                                 name=f"vpad{p}") for p in range(8)]
            vt = [persist.tile([128, DH], BF16, tag=f"vt{m}", name=f"vt{m}")
                  for m in range(7)]
            osum = [persist.tile([128, N], BF16, tag=f"osum{p}",
                                 name=f"osum{p}") for p in range(8)]

            a2tap_cm = tc.tile_pool(name="a2tap", bufs=1)
            a2tap = a2tap_cm.__enter__()
            a2ta = [a2tap.tile([128, NGA * 128], BF16, tag=f"a2ta{m}",
                               name=f"a2ta{m}") for m in range(7)]
            a2tb = [a2tap.tile([128, (NGB + NGC) * 128], BF16, tag=f"a2tb{m}",
                               name=f"a2tb{m}") for m in range(7)]
            with tc.tile_pool(name="qk", bufs=1) as qkpool:
              # =========== Phase A: projections ============================
              with (
                tc.tile_pool(name="pa", bufs=3, space="PSUM") as pa,
                tc.tile_pool(name="ax", bufs=1) as axpool,
              ):
                x_w = axpool.tile([128, 3 * N], BF16, tag="xw", name="xw")
                for k in range(3):
                    nc.gpsimd.dma_start(x_w[:, k * N:(k + 1) * N],
                                        x_c.ap()[:, k * N:(k + 1) * N])
                x_t = [x_w[:, k * N:(k + 1) * N] for k in range(3)]

                def load_a(name, src_ap, shape, dt=F32):
                    t = axpool.tile(shape, dt, tag=name, name=name)
                    nc.sync.dma_start(t[:], src_ap)
                    return t

                wq_w = load_a("wq_w", wq3.ap()[:], [128, 3 * NH_KD], BF16)
                wk_w = load_a("wk_w", wk3.ap()[:], [128, 3 * NH_KD], BF16)
                wv_w = load_a("wv_w", wv3.ap()[:], [128, 3 * DH], BF16)
                wq_t = [wq_w[:, k * NH_KD:(k + 1) * NH_KD] for k in range(3)]
                wk_t = [wk_w[:, k * NH_KD:(k + 1) * NH_KD] for k in range(3)]
                wv_t = [wv_w[:, k * DH:(k + 1) * DH] for k in range(3)]
                load_small_consts()

                q_t = [axpool.tile([128, N], BF16, tag=f"q{k}", name=f"q{k}")
                       for k in range(2)]
                k_t = [qkpool.tile([128, N], BF16, tag=f"k{k}", name=f"kp{k}")
                       for k in range(2)]
                q2_t = [qkpool.tile([128, NG * 128], BF16, tag=f"q2{k}",
                                    name=f"q2{k}") for k in range(2)]

                # q and k projections: out[ot*128.., n]
                for (wts, bias, dst) in ((wq_t, bq_t, q_t), (wk_t, bk_t, k_t)):
                    for ot in range(2):
                        ps = pa.tile([128, 1024], F32, tag="pa", name="pa")
                        for ci, chs in enumerate((CH0, CH1)):
                            pchunk = ps[:, PS0] if ci == 0 else ps[:, PS1]
                            for kt in range(3):
                                nc.tensor.matmul(
                                    pchunk,
                                    lhsT=wts[kt][:, ot * 128:(ot + 1) * 128],
                                    rhs=x_t[kt][:, chs],
                                    start=(kt == 0), stop=(kt == 2))
                        nc.vector.tensor_scalar_add(dst[ot][:], psum2view(ps),
                                                    bias[ot])

                # Q2: 8 per-head-scaled copies of q, bf16
                for kt in range(2):
                    qv = q_t[kt][:].rearrange("p (a i) -> p a i", i=16)
                    q2v = q2_t[kt][:].rearrange("p (a g i) -> p a g i",
                                                g=8, i=16)
                    for (a0, a1) in ((0, 16), (16, NG)):
                        for g in range(8):
                            nc.vector.tensor_scalar_mul(
                                q2v[:, a0:a1, g, :], qv[:, a0:a1, :],
                                sq_t[kt][:, g:g + 1])

                # v projection straight into the zero-padded 30x30 grid
                for p in range(8):
                    vvz = vpad[p][:].rearrange("p (r c) -> p r c", c=30)
                    nc.gpsimd.memset(vvz[:, 0, :], 0.0)
                    nc.gpsimd.memset(vvz[:, 29, :], 0.0)
                    nc.gpsimd.memset(vvz[:, 1:29, 0], 0.0)
                    nc.gpsimd.memset(vvz[:, 1:29, 29], 0.0)
                    ps = pa.tile([128, 1024], F32, tag="pa", name="pa")
                    for ci in range(2):
                        pchunk = ps[:, PS0] if ci == 0 else ps[:, PS1]
                        for kt in range(3):
                            nc.tensor.matmul(
                                pchunk,
                                lhsT=wv_t[kt][:, p * 128:(p + 1) * 128],
                                rhs=x_t[kt][:, CH0 if ci == 0 else CH1],
                                start=(kt == 0), stop=(kt == 2))
                    vview = vpad[p][:].rearrange("p (r c) -> p r c", c=30)
                    rows = vview[:, 1:29, 1:29].rearrange(
                        "p (a r) c -> p a r c", a=2)
                    pin = psum2view(ps).rearrange("p a (r c) -> p a r c", c=28)
                    nc.scalar.activation(rows, pin, AF.Identity, bias=bv_t[p])

                # V^T tiles [m,(g,d)] directly from the projection (+bias via
                # a K=1 ones-row matmul)
                nc.gpsimd.memset(vt[6][:], 0.0)
                for mt in range(7):
                    M = 128 if mt < 6 else 16
                    msl = slice(mt * 128, mt * 128 + M)
                    ps = pa.tile([128, 1024], F32, tag="pa", name="pa")
                    for ci in range(2):
                        pchunk = ps[0:M, ci * 512:(ci + 1) * 512]
                        csl = slice(ci * 512, (ci + 1) * 512)
                        for kt in range(3):
                            nc.tensor.matmul(pchunk,
                                             lhsT=x_t[kt][:, msl],
                                             rhs=wv_t[kt][:, csl],
                                             start=(kt == 0), stop=(kt == 2))
                        nc.scalar.copy(vt[mt][0:M, csl], pchunk)

              # =========== Phase C: attention per 16-query group ===========
              with (
                  tc.tile_pool(name="pc", bufs=2, space="PSUM") as pc,
                  tc.tile_pool(name="cw", bufs=2) as cw,
                  tc.tile_pool(name="cz", bufs=3) as cz,
              ):
                  abt_v = abt.ap().rearrange("(a p) c -> p a c", p=128)
                  abtiles = {}
                  pending = []
                  a2cur = [None]

                  def fetch_ab(k):
                      nab = min(4, NG - k * 4)
                      t = cw.tile([128, 4 * N], FP8, tag="ab", name="ab",
                                  bufs=5)
                      nc.scalar.dma_start(
                          t[:, 0:nab * N].rearrange("p (a c) -> p a c", c=N),
                          abt_v[:, k * 4:k * 4 + nab, :])
                      abtiles[k] = t

                  for _k in range(4):
                      fetch_ab(_k)

                  zpad = cw.tile([128, 28 * 112], BF16, tag="zpad",
                                 name="zpad", bufs=1)
                  nc.gpsimd.memset(zpad[:], 0.0)
                  zv = zpad[:].rearrange("p (a c) -> p a c", c=112)
                  for (tns, ngr) in ((a2da, NGA), (a2db, NGB), (a2dc, NGC)):
                      dvz = tns.ap().rearrange("(a p) c -> p a c", p=128)
                      nc.gpsimd.dma_start(dvz[:, :, 784:896],
                                          zv[:, 0:ngr, :])
                  for gi in range(NG):
                      gsl = slice(gi * 128, (gi + 1) * 128)
                      if gi % 4 == 2 and gi // 4 + 4 <= (NG - 1) // 4:
                          fetch_ab(gi // 4 + 4)
                      ab4 = abtiles[gi // 4]
                      ab = ab4[:, (gi % 4) * N:(gi % 4 + 1) * N]

                      lg = pc.tile([128, 1024], F32, tag="lg", name="lg",
                                   bufs=3)
                      for ci, chs in enumerate((CH0, CH1)):
                          pchunk = lg[:, PS0] if ci == 0 else lg[:, PS1]
                          for kt in range(2):
                              nc.tensor.matmul(pchunk,
                                               lhsT=q2_t[kt][:, gsl],
                                               rhs=k_t[kt][:, chs],
                                               start=(kt == 0), stop=False)
                          nc.tensor.matmul(pchunk, lhsT=identf8_t[:],
                                           rhs=ab[:, chs],
                                           start=False, stop=True)

                      e = cw.tile([128, N], BF16, tag="e", name="e",
                                  bufs=3)
                      z = cz.tile([128, 1], F32, tag="z", name="z")
                      nc.scalar.activation(e[:], psum2view(lg), AF.Exp,
                                           bias=bth1_t, accum_out=z[:])

                      r = cz.tile([128, 1], F32, tag="r", name="r")
                      nc.vector.reciprocal(r[:], z[:])
                      w2s = cz.tile([128, 128], BF16, tag="w2s", name="w2s")
                      nc.vector.tensor_scalar_mul(w2s[:], w2bd_t[:], r[:])

                      pending.append((gi, e, w2s))
                      if gi == NG - 1:
                          flush = pending
                          pending = []
                      elif len(pending) > 2:
                          flush = [pending.pop(0)]
                      else:
                          flush = []
                      for (fgi, fe, fw2s) in flush:
                          a2p = pc.tile([128, 1024], F32, tag="a2p",
                                        name="a2p", bufs=1)
                          for ci, chs in enumerate((CH0, CH1)):
                              pchunk = a2p[:, PS0] if ci == 0 else a2p[:, PS1]
                              nc.tensor.matmul(pchunk, lhsT=fw2s[:],
                                               rhs=fe[:, chs],
                                               start=True, stop=True)
                          if fgi % 4 == 0:
                              a2w = cw.tile([128, 4 * 896], BF16, tag="a2",
                                            name="a2")
                              a2wv = a2w[:].rearrange("p (a c) -> p a c",
                                                      c=896)
                              a2cur[0] = a2wv
                          j = fgi % 4
                          nc.vector.tensor_scalar_add(
                              a2cur[0][:, j, 0:784], psum2view(a2p), bth2_t)
                          if j == 3 or fgi == NG - 1:
                              gi0 = fgi - j
                              nab = j + 1
                              if gi0 < NGA:
                                  dv = a2da.ap().rearrange("(a p) c -> p a c",
                                                           p=128)
                                  dst = dv[:, gi0:gi0 + nab, :]
                              elif gi0 < NGA + NGB:
                                  dv = a2db.ap().rearrange("(a p) c -> p a c",
                                                           p=128)
                                  dst = dv[:, gi0 - NGA:gi0 - NGA + nab, :]
                              else:
                                  dv = a2dc.ap().rearrange("(a p) c -> p a c",
                                                           p=128)
                                  g0c = gi0 - NGA - NGB
                                  dst = dv[:, g0c:g0c + nab, :]
                              nc.gpsimd.dma_start(dst[:, :, 0:784],
                                                  a2cur[0][:, 0:nab, 0:784])
                          if fgi >= NGA - 1 and (fgi - (NGA - 1)) % 3 == 0:
                              mt = (fgi - (NGA - 1)) // 3
                              if mt < 7:
                                  nc.sync.dma_start_transpose(
                                      a2ta[mt][:],
                                      a2da.ap()[:, mt * 128:(mt + 1) * 128])
                          if fgi >= NGA + NGB - 1:
                              mtb = fgi - (NGA + NGB - 1)
                              if mtb < 7:
                                  nc.sync.dma_start_transpose(
                                      a2tb[mtb][:, 0:NGB * 128],
                                      a2db.ap()[:, mtb * 128:(mtb + 1) * 128])

            # ======= Phase D: attn@v + depthwise conv, fused projection ====
            # Two passes over output n-chunks: chunk0 (rows 0..15, 448 cols =
            # groups 0..27 = the a2da half, transposed during phase C) and
            # chunk1 (rows 16..27, 336 cols = groups 28..48 = a2db).
            with (
                tc.tile_pool(name="pd", bufs=2, space="PSUM") as pd,
                tc.tile_pool(name="pe", bufs=1, space="PSUM") as pe,
                tc.tile_pool(name="dg", bufs=2) as dgp,
                tc.tile_pool(name="ow", bufs=1) as ow,
            ):
                nc.sync.dma_start(wp_w[:], wp8.ap()[:])
                # b-half transposes not yet issued inside phase C (the loop
                # only reaches mtb = NG-1 - (NGA+NGB-1))
                for mtb in range(NG - (NGA + NGB - 1), 7):
                    nc.sync.dma_start_transpose(
                        a2tb[mtb][:, 0:NGB * 128],
                        a2db.ap()[:, mtb * 128:(mtb + 1) * 128])
                for mt in range(7):
                    nc.sync.dma_start_transpose(
                        a2tb[mt][:, NGB * 128:(NGB + NGC) * 128],
                        a2dc.ap()[:, mt * 128:(mt + 1) * 128])

                ot = [ow.tile([128, N], F32, tag=f"ot{mt}", name=f"ot{mt}")
                      for mt in range(3)]
                DCH = ((0, 16, 28, 448), (16, 12, 21, 336))
                for ci, (r0, nr, ngr, w) in enumerate(DCH):
                    csl = slice(0, 448) if ci == 0 else slice(448, 784)
                    half = a2ta if ci == 0 else a2tb
                    pp = [pe.tile([128, w], F32, tag=f"pp{ci}{mt}",
                                  name=f"pp{ci}{mt}") for mt in range(3)]
                    prev_e = None
                    for g in range(8):
                        po = pd.tile([128, w], F32, tag="po", name="po")
                        dgt = [dgp.tile([128, 128], BF16, tag="dg", name="dg")
                               for _ in range(9)]
                        for t in range(9):
                            nc.vector.tensor_scalar_mul(
                                dgt[t][:], ident_t[:], wtap_t[g][:, t:t + 1])
                        vv = vpad[g][:].rearrange("p (r c) -> p r c", c=30)
                        for t in range(9):
                            dy, dx = t // 3, t % 3
                            srcv = vv[:, r0 + dy:r0 + dy + nr, dx:dx + 28]
                            nc.tensor.matmul(po[:], lhsT=dgt[t][:], rhs=srcv,
                                             start=(t == 0), stop=False)
                        for mt in range(7):
                            cols = half[mt][:].rearrange(
                                "p (a G i) -> p a G i", G=8, i=16)[:, :, g, :]
                            nc.tensor.matmul(
                                po[:],
                                lhsT=vt[mt][:, g * 128:(g + 1) * 128],
                                rhs=cols, start=False, stop=(mt == 6))
                        nc.scalar.activation(osum[g][:, csl], po[:],
                                             AF.Identity, bias=bdw_t[g])
                        if prev_e is not None:
                            for mt in range(3):
                                nc.tensor.matmul(
                                    pp[mt][:],
                                    lhsT=wp_t[prev_e][:,
                                                      mt * 128:(mt + 1) * 128],
                                    rhs=osum[prev_e][:, csl],
                                    start=(prev_e == 0), stop=False)
                        prev_e = g
                    for mt in range(3):
                        nc.tensor.matmul(
                            pp[mt][:],
                            lhsT=wp_t[7][:, mt * 128:(mt + 1) * 128],
                            rhs=osum[7][:, csl],
                            start=False, stop=True)
                    for mt in range(3):
                        nc.scalar.activation(ot[mt][:, csl], pp[mt][:],
                                             AF.Identity, bias=bp_t[mt])
                for mt in range(3):
                    nc.scalar.dma_start(
                        out.ap()[mt * 128:(mt + 1) * 128, :], ot[mt][:])

            a2tap_cm.__exit__(None, None, None)

    nc.compile()
    return nc


def _prep_common(inputs):
    f32 = np.float32
    scale = np.float32(KD ** -0.5)
    q_s, q_b = inputs["q_s"], inputs["q_b"]
    k_s, k_b = inputs["k_s"], inputs["k_b"]
    v_s, v_b = inputs["v_s"], inputs["v_b"]
    p_s, p_b = inputs["p_s"], inputs["p_b"]

    Wq = np.asarray(inputs["Wq"], f32) * np.asarray(q_s, f32)[:, None] * scale
    bqv = (np.asarray(q_s, f32) * np.asarray(inputs["bq"], f32)
           + np.asarray(q_b, f32)) * scale
    Wk = np.asarray(inputs["Wk"], f32) * np.asarray(k_s, f32)[:, None]
    bkv = np.asarray(k_s, f32) * np.asarray(inputs["bk"], f32) + np.asarray(k_b, f32)
    Wv = np.asarray(inputs["Wv"], f32) * np.asarray(v_s, f32)[:, None]
    bvv = np.asarray(v_s, f32) * np.asarray(inputs["bv"], f32) + np.asarray(v_b, f32)
    Wp = np.asarray(inputs["Wp"], f32) * np.asarray(p_s, f32)[:, None]
    bpv = np.asarray(p_s, f32) * np.asarray(inputs["bp"], f32) + np.asarray(p_b, f32)

    Wth1 = np.asarray(inputs["Wth1"], f32)
    bth1 = np.asarray(inputs["bth1"], f32)
    Wth2 = np.asarray(inputs["Wth2"], f32)
    bth2 = np.asarray(inputs["bth2"], f32)

    # talking-head-1 folded bias table, rows ordered (group, g, i); bth1 is
    # applied separately as the ACT exp bias
    ab1 = Wth1 @ np.asarray(inputs["attention_biases"], f32)      # [8, 784]
    idx = np.asarray(inputs["bias_idxs"])                          # [784, 784]
    ab_full = ab1[:, idx]                                          # [8,784,784]
    abt = np.ascontiguousarray(
        ab_full.reshape(8, NG, 16, N).transpose(1, 0, 2, 3)
    ).reshape(NG * 128, N).astype(F8)

    # depthwise weights folded with BN
    wvl = np.asarray(inputs["Wvl"], f32)[:, 0, :, :].reshape(DH, 9)
    vl_s = np.asarray(inputs["vl_s"], f32)
    wtap = wvl * vl_s[:, None]
    bdw = (np.asarray(inputs["bvl"], f32) * vl_s
           + np.asarray(inputs["vl_b"], f32))

    def ktile_pack(wT, nk):
        # [nk*128, C] -> [128, nk*C] with k-tile-major free dim
        C = wT.shape[1]
        return np.ascontiguousarray(
            wT.reshape(nk, 128, C).transpose(1, 0, 2).reshape(128, nk * C))

    sqv = np.repeat(Wth1.T, KD, axis=0).astype(f32)                # [256, 8]
    vecs = np.zeros((128, 113), f32)
    vecs[:, 0:2] = bqv.reshape(2, 128).T
    vecs[:, 2:4] = bkv.reshape(2, 128).T
    vecs[:, 4:12] = bvv.reshape(8, 128).T
    s2 = Wth2.sum(axis=1) + N * bth2                   # [8] per out-head
    bdw2 = bdw + bvv * np.repeat(s2, D)
    vecs[:, 12:20] = bdw2.reshape(8, 128).T
    vecs[:, 20:23] = bpv.reshape(3, 128).T
    vecs[:, 23] = np.repeat(bth1, 16)
    vecs[:, 24] = np.repeat(bth2, 16)
    vecs[:, 25:33] = sqv[0:128]
    vecs[:, 33:41] = sqv[128:256]
    for g in range(8):
        vecs[:, 41 + g * 9:50 + g * 9] = wtap[g * 128:(g + 1) * 128]

    common = {
        "wq3": ktile_pack(np.ascontiguousarray(Wq.T), 3).astype(BF),
        "wk3": ktile_pack(np.ascontiguousarray(Wk.T), 3).astype(BF),
        "wv3": ktile_pack(np.ascontiguousarray(Wv.T), 3).astype(BF),
        "wp8": ktile_pack(np.ascontiguousarray(Wp.T), 8).astype(BF),
        "vecs": vecs,
        "w2bd": np.kron(Wth2.T, np.eye(16, dtype=f32)).astype(f32),
        "ident": np.eye(128, dtype=f32).astype(BF),
        "identf8": np.eye(128, dtype=f32).astype(F8),
        "abt": abt,
    }
    return common


def kernel(**inputs):
    global LAST_RESULTS
    if "nc" not in _CACHE:
        _CACHE["nc"] = _build_program()
    nc = _CACHE["nc"]

    common = _prep_common(inputs)
    x = np.asarray(inputs["x"], np.float32)          # [8, 384, 28, 28]
    in_maps = []
    for c in range(B):
        m = dict(common)
        xc = x[c].reshape(3, 128, N).transpose(1, 0, 2).reshape(128, 3 * N)
        m["x_c"] = np.ascontiguousarray(xc).astype(BF)
        in_maps.append(m)

    import os
    trace = bool(int(os.environ.get("KERNEL_TRACE", "0")))
    res = run_bass_kernel_spmd(nc, in_maps, core_ids=list(range(B)),
                               trace=trace)
    LAST_RESULTS = res
    out = np.stack([res.results[c]["out"].reshape(DIM, RES, RES)
                    for c in range(B)])
    return out.astype(np.float32)


# revision 45
# speedup vs baseline: 1.0535x; 1.0535x over previous
"""Trainium2 Bass kernel for nn_Attention4D (EfficientViT-style attention).

Strategy (8 NeuronCores, data-parallel over batch B=8, one batch element per core):
  - BN folded into conv weights on host.
  - Talking-head-1 folded into per-head-scaled queries (Q2), so q@k contracts
    256 channels at full PE efficiency and th1 disappears.
  - Attention tensor layout: partition dim = (head, 16 queries) interleaved
    (49 groups of 128 partitions), free dim = keys m (784).  Softmax is then
    per-partition (ACT exp with fused accumulate for the denominator; bth1
    rides the free per-partition ACT bias), and talking-head-2 is one
    block-diagonal 128x128 PE matmul per group with the softmax normalization
    folded into the block-diag weights.
  - Relative-position biases are a precomputed fp8 table accumulated into the
    logits PSUM with an identity matmul.
  - attn@v needs keys on partitions, so A2 makes one DRAM round trip through
    dma_start_transpose (two scratch tensors so the read-back overlaps the
    tail of the softmax phase).  V^T is produced directly by the projection.
  - The 3x3 depthwise conv (v_local) runs on the PE as 9 diagonal-weight
    matmuls accumulated into the same PSUM banks as attn@v output.
"""

import sys

sys.path.insert(0, "/opt/trn_rl_repo")

import numpy as np
import ml_dtypes

import concourse.bass as bass
import concourse.tile as tile
from concourse import bacc, mybir
from concourse.bass_utils import run_bass_kernel_spmd

F32 = mybir.dt.float32
BF16 = mybir.dt.bfloat16
FP8 = mybir.dt.float8e4
AF = mybir.ActivationFunctionType
BF = ml_dtypes.bfloat16
F8 = ml_dtypes.float8_e4m3

HEADS, KD, AR, RES, DIM = 8, 32, 4, 28, 384
D = AR * KD            # 128
DH = HEADS * D         # 1024
NH_KD = HEADS * KD     # 256
N = RES * RES          # 784
NG = N // 16           # 49 groups of 16 queries
NGA = 28               # groups in first scratch tensor (output chunk 0)
NGB = 16               # second tensor (groups 28..43)
NGC = NG - NGA - NGB   # third tensor (groups 44..48)
B = 8

_CACHE = {}
LAST_RESULTS = None  # test.py reads exec_time from here


def _build_program():
    nc = bacc.Bacc("TRN2", target_bir_lowering=False, debug=False,
                   enable_asserts=True)

    def din(name, shape, dt=F32):
        return nc.dram_tensor(name, shape, dt, kind="ExternalInput")

    x_c = din("x_c", [128, 3 * N], BF16)
    wq3 = din("wq3", [128, 3 * NH_KD], BF16)
    wk3 = din("wk3", [128, 3 * NH_KD], BF16)
    wv3 = din("wv3", [128, 3 * DH], BF16)
    wp8 = din("wp8", [128, 8 * DIM], BF16)
    vecs = din("vecs", [128, 113])
    w2bd = din("w2bd", [128, 128])
    ident = din("ident", [128, 128], BF16)
    identf8 = din("identf8", [128, 128], FP8)
    abt = din("abt", [NG * 128, N], FP8)

    out = nc.dram_tensor("out", [DIM, N], F32, kind="ExternalOutput")
    a2da = nc.dram_tensor("a2da", [NGA * 128, 896], BF16, kind="Internal")
    a2db = nc.dram_tensor("a2db", [NGB * 128, 896], BF16, kind="Internal")
    a2dc = nc.dram_tensor("a2dc", [NGC * 128, 896], BF16, kind="Internal")

    CH0 = slice(0, 392)          # logits free-dim chunks (within 2-bank psum)
    CH1 = slice(392, 784)
    PS0 = slice(0, 392)          # psum [128,1024]: bank0
    PS1 = slice(512, 904)        # bank1

    def psum2view(ps):
        # [128, 2, 392] view of a 2-bank psum tile's used region
        return ps[:].rearrange("p (a c) -> p a c", c=512)[:, :, 0:392]

    with tile.TileContext(nc) as tc:
        with (
            tc.tile_pool(name="consts", bufs=1) as consts,
            tc.tile_pool(name="persist", bufs=1) as persist,
        ):
            # ---- resident weights/constants -------------------------------
            def load_const(name, src_ap, shape, dt=F32):
                t = consts.tile(shape, dt, tag=name, name=name)
                nc.sync.dma_start(t[:], src_ap)
                return t

            wp_w = consts.tile([128, 8 * DIM], BF16, tag="wp_w", name="wp_w")
            wp_t = [wp_w[:, k * DIM:(k + 1) * DIM] for k in range(8)]
            vec_t = consts.tile([128, 113], F32, tag="vec_t", name="vec_t")
            bq_t = [vec_t[:, k:k + 1] for k in range(2)]
            bk_t = [vec_t[:, 2 + k:3 + k] for k in range(2)]
            bv_t = [vec_t[:, 4 + k:5 + k] for k in range(8)]
            bdw_t = [vec_t[:, 12 + k:13 + k] for k in range(8)]
            bp_t = [vec_t[:, 20 + k:21 + k] for k in range(3)]
            bth1_t = vec_t[:, 23:24]
            bth2_t = vec_t[:, 24:25]
            sq_t = [vec_t[:, 25 + k * 8:33 + k * 8] for k in range(2)]
            wtap_t = [vec_t[:, 41 + g * 9:50 + g * 9] for g in range(8)]
            w2bd_t = consts.tile([128, 128], F32, tag="w2bd", name="w2bd")
            ident_t = consts.tile([128, 128], BF16, tag="ident", name="ident")
            identf8_t = consts.tile([128, 128], FP8, tag="identf8",
                                    name="identf8")

            def load_small_consts():
                nc.sync.dma_start(vec_t[:], vecs.ap()[:])
                nc.sync.dma_start(w2bd_t[:], w2bd.ap()[:])
                nc.sync.dma_start(ident_t[:], ident.ap()[:])
                nc.sync.dma_start(identf8_t[:], identf8.ap()[:])

            # ---- persistent activations -----------------------------------
            vpad = [persist.tile([128, 900], BF16, tag=f"vpad{p}",
                                 name=f"vpad{p}") for p in range(8)]
            vt = [persist.tile([128, DH], BF16, tag=f"vt{m}", name=f"vt{m}")
                  for m in range(7)]
            osum = [persist.tile([128, N], BF16, tag=f"osum{p}",
                                 name=f"osum{p}") for p in range(8)]

            a2tap_cm = tc.tile_pool(name="a2tap", bufs=1)
            a2tap = a2tap_cm.__enter__()
            a2ta = [a2tap.tile([128, NGA * 128], BF16, tag=f"a2ta{m}",
                               name=f"a2ta{m}") for m in range(7)]
            a2tb = [a2tap.tile([128, (NGB + NGC) * 128], BF16, tag=f"a2tb{m}",
                               name=f"a2tb{m}") for m in range(7)]
            with tc.tile_pool(name="qk", bufs=1) as qkpool:
              # =========== Phase A: projections ============================
              with (
                tc.tile_pool(name="pa", bufs=3, space="PSUM") as pa,
                tc.tile_pool(name="ax", bufs=1) as axpool,
              ):
                x_w = axpool.tile([128, 3 * N], BF16, tag="xw", name="xw")
                for k in range(3):
                    nc.gpsimd.dma_start(x_w[:, k * N:(k + 1) * N],
                                        x_c.ap()[:, k * N:(k + 1) * N])
                x_t = [x_w[:, k * N:(k + 1) * N] for k in range(3)]

                def load_a(name, src_ap, shape, dt=F32):
                    t = axpool.tile(shape, dt, tag=name, name=name)
                    nc.sync.dma_start(t[:], src_ap)
                    return t

                wq_w = load_a("wq_w", wq3.ap()[:], [128, 3 * NH_KD], BF16)
                wk_w = load_a("wk_w", wk3.ap()[:], [128, 3 * NH_KD], BF16)
                wv_w = load_a("wv_w", wv3.ap()[:], [128, 3 * DH], BF16)
                wq_t = [wq_w[:, k * NH_KD:(k + 1) * NH_KD] for k in range(3)]
                wk_t = [wk_w[:, k * NH_KD:(k + 1) * NH_KD] for k in range(3)]
                wv_t = [wv_w[:, k * DH:(k + 1) * DH] for k in range(3)]
                load_small_consts()

                q_t = [axpool.tile([128, N], BF16, tag=f"q{k}", name=f"q{k}")
                       for k in range(2)]
                k_t = [qkpool.tile([128, N], BF16, tag=f"k{k}", name=f"kp{k}")
                       for k in range(2)]
                q2_t = [qkpool.tile([128, NG * 128], BF16, tag=f"q2{k}",
                                    name=f"q2{k}") for k in range(2)]

                # q and k projections: out[ot*128.., n]
                for (wts, bias, dst) in ((wq_t, bq_t, q_t), (wk_t, bk_t, k_t)):
                    for ot in range(2):
                        ps = pa.tile([128, 1024], F32, tag="pa", name="pa")
                        for ci, chs in enumerate((CH0, CH1)):
                            pchunk = ps[:, PS0] if ci == 0 else ps[:, PS1]
                            for kt in range(3):
                                nc.tensor.matmul(
                                    pchunk,
                                    lhsT=wts[kt][:, ot * 128:(ot + 1) * 128],
                                    rhs=x_t[kt][:, chs],
                                    start=(kt == 0), stop=(kt == 2))
                        nc.vector.tensor_scalar_add(dst[ot][:], psum2view(ps),
                                                    bias[ot])

                # Q2: 8 per-head-scaled copies of q, bf16
                for kt in range(2):
                    qv = q_t[kt][:].rearrange("p (a i) -> p a i", i=16)
                    q2v = q2_t[kt][:].rearrange("p (a g i) -> p a g i",
                                                g=8, i=16)
                    for (a0, a1) in ((0, 16), (16, NG)):
                        for g in range(8):
                            nc.vector.tensor_scalar_mul(
                                q2v[:, a0:a1, g, :], qv[:, a0:a1, :],
                                sq_t[kt][:, g:g + 1])

                # v projection straight into the zero-padded 30x30 grid
                for p in range(8):
                    vvz = vpad[p][:].rearrange("p (r c) -> p r c", c=30)
                    nc.gpsimd.memset(vvz[:, 0, :], 0.0)
                    nc.gpsimd.memset(vvz[:, 29, :], 0.0)
                    nc.gpsimd.memset(vvz[:, 1:29, 0], 0.0)
                    nc.gpsimd.memset(vvz[:, 1:29, 29], 0.0)
                    ps = pa.tile([128, 1024], F32, tag="pa", name="pa")
                    for ci in range(2):
                        pchunk = ps[:, PS0] if ci == 0 else ps[:, PS1]
                        for kt in range(3):
                            nc.tensor.matmul(
                                pchunk,
                                lhsT=wv_t[kt][:, p * 128:(p + 1) * 128],
                                rhs=x_t[kt][:, CH0 if ci == 0 else CH1],
                                start=(kt == 0), stop=(kt == 2))
                    vview = vpad[p][:].rearrange("p (r c) -> p r c", c=30)
                    rows = vview[:, 1:29, 1:29].rearrange(
                        "p (a r) c -> p a r c", a=2)
                    pin = psum2view(ps).rearrange("p a (r c) -> p a r c", c=28)
                    nc.scalar.activation(rows, pin, AF.Identity, bias=bv_t[p])

                # V^T tiles [m,(g,d)] directly from the projection (+bias via
                # a K=1 ones-row matmul)
                nc.gpsimd.memset(vt[6][:], 0.0)
                for mt in range(7):
                    M = 128 if mt < 6 else 16
                    msl = slice(mt * 128, mt * 128 + M)
                    ps = pa.tile([128, 1024], F32, tag="pa", name="pa")
                    for ci in range(2):
                        pchunk = ps[0:M, ci * 512:(ci + 1) * 512]
                        csl = slice(ci * 512, (ci + 1) * 512)
                        for kt in range(3):
                            nc.tensor.matmul(pchunk,
                                             lhsT=x_t[kt][:, msl],
                                             rhs=wv_t[kt][:, csl],
                                             start=(kt == 0), stop=(kt == 2))
                        nc.scalar.copy(vt[mt][0:M, csl], pchunk)

              # =========== Phase C: attention per 16-query group ===========
              with (
                  tc.tile_pool(name="pc", bufs=2, space="PSUM") as pc,
                  tc.tile_pool(name="cw", bufs=2) as cw,
                  tc.tile_pool(name="cz", bufs=3) as cz,
              ):
                  abt_v = abt.ap().rearrange("(a p) c -> p a c", p=128)
                  abtiles = {}
                  pending = []
                  a2cur = [None]

                  def fetch_ab(k):
                      nab = min(4, NG - k * 4)
                      t = cw.tile([128, 4 * N], FP8, tag="ab", name="ab",
                                  bufs=5)
                      nc.scalar.dma_start(
                          t[:, 0:nab * N].rearrange("p (a c) -> p a c", c=N),
                          abt_v[:, k * 4:k * 4 + nab, :])
                      abtiles[k] = t

                  for _k in range(4):
                      fetch_ab(_k)

                  zpad = cw.tile([128, 28 * 112], BF16, tag="zpad",
                                 name="zpad", bufs=1)
                  nc.gpsimd.memset(zpad[:], 0.0)
                  zv = zpad[:].rearrange("p (a c) -> p a c", c=112)
                  for (tns, ngr) in ((a2da, NGA), (a2db, NGB), (a2dc, NGC)):
                      dvz = tns.ap().rearrange("(a p) c -> p a c", p=128)
                      nc.gpsimd.dma_start(dvz[:, :, 784:896],
                                          zv[:, 0:ngr, :])
                  for gi in range(NG):
                      gsl = slice(gi * 128, (gi + 1) * 128)
                      if gi % 4 == 2 and gi // 4 + 4 <= (NG - 1) // 4:
                          fetch_ab(gi // 4 + 4)
                      ab4 = abtiles[gi // 4]
                      ab = ab4[:, (gi % 4) * N:(gi % 4 + 1) * N]

                      lg = pc.tile([128, 1024], F32, tag="lg", name="lg",
                                   bufs=2)
                      for ci, chs in enumerate((CH0, CH1)):
                          pchunk = lg[:, PS0] if ci == 0 else lg[:, PS1]
                          for kt in range(2):
                              nc.tensor.matmul(pchunk,
                                               lhsT=q2_t[kt][:, gsl],
                                               rhs=k_t[kt][:, chs],
                                               start=(kt == 0), stop=False)
                          nc.tensor.matmul(pchunk, lhsT=identf8_t[:],
                                           rhs=ab[:, chs],
                                           start=False, stop=True)

                      e = cw.tile([128, N], BF16, tag="e", name="e",
                                  bufs=3)
                      z = cz.tile([128, 1], F32, tag="z", name="z")
                      nc.scalar.activation(e[:], psum2view(lg), AF.Exp,
                                           bias=bth1_t, accum_out=z[:])

                      r = cz.tile([128, 1], F32, tag="r", name="r")
                      nc.vector.reciprocal(r[:], z[:])
                      w2s = cz.tile([128, 128], BF16, tag="w2s", name="w2s")
                      nc.vector.tensor_scalar_mul(w2s[:], w2bd_t[:], r[:])

                      pending.append((gi, e, w2s))
                      if gi == NG - 1:
                          flush = pending
                          pending = []
                      elif len(pending) > 2:
                          flush = [pending.pop(0)]
                      else:
                          flush = []
                      for (fgi, fe, fw2s) in flush:
                          a2p = pc.tile([128, 1024], F32, tag="a2p",
                                        name="a2p", bufs=2)
                          for ci, chs in enumerate((CH0, CH1)):
                              pchunk = a2p[:, PS0] if ci == 0 else a2p[:, PS1]
                              nc.tensor.matmul(pchunk, lhsT=fw2s[:],
                                               rhs=fe[:, chs],
                                               start=True, stop=True)
                          if fgi % 4 == 0:
                              a2w = cw.tile([128, 4 * 896], BF16, tag="a2",
                                            name="a2")
                              a2wv = a2w[:].rearrange("p (a c) -> p a c",
                                                      c=896)
                              a2cur[0] = a2wv
                          j = fgi % 4
                          nc.vector.tensor_scalar_add(
                              a2cur[0][:, j, 0:784], psum2view(a2p), bth2_t)
                          if j == 3 or fgi == NG - 1:
                              gi0 = fgi - j
                              nab = j + 1
                              if gi0 < NGA:
                                  dv = a2da.ap().rearrange("(a p) c -> p a c",
                                                           p=128)
                                  dst = dv[:, gi0:gi0 + nab, :]
                              elif gi0 < NGA + NGB:
                                  dv = a2db.ap().rearrange("(a p) c -> p a c",
                                                           p=128)
                                  dst = dv[:, gi0 - NGA:gi0 - NGA + nab, :]
                              else:
                                  dv = a2dc.ap().rearrange("(a p) c -> p a c",
                                                           p=128)
                                  g0c = gi0 - NGA - NGB
                                  dst = dv[:, g0c:g0c + nab, :]
                              nc.gpsimd.dma_start(dst[:, :, 0:784],
                                                  a2cur[0][:, 0:nab, 0:784])
                          if fgi >= NGA - 1 and (fgi - (NGA - 1)) % 3 == 0:
                              mt = (fgi - (NGA - 1)) // 3
                              if mt < 7:
                                  nc.sync.dma_start_transpose(
                                      a2ta[mt][:],
                                      a2da.ap()[:, mt * 128:(mt + 1) * 128])
                          pass

            # ======= Phase D: attn@v + depthwise conv, fused projection ====
            # Two passes over output n-chunks: chunk0 (rows 0..15, 448 cols =
            # groups 0..27 = the a2da half, transposed during phase C) and
            # chunk1 (rows 16..27, 336 cols = groups 28..48 = a2db).
            with (
                tc.tile_pool(name="pd", bufs=2, space="PSUM") as pd,
                tc.tile_pool(name="dg", bufs=4) as dgp,
                tc.tile_pool(name="pe", bufs=1, space="PSUM") as pe,
                tc.tile_pool(name="ow", bufs=1) as ow,
            ):
                nc.sync.dma_start(wp_w[:], wp8.ap()[:])
                # b-half transposes not yet issued inside phase C (the loop
                # only reaches mtb = NG-1 - (NGA+NGB-1))
                for mtb in range(0, 7):
                    nc.sync.dma_start_transpose(
                        a2tb[mtb][:, 0:NGB * 128],
                        a2db.ap()[:, mtb * 128:(mtb + 1) * 128])
                for mt in range(7):
                    nc.sync.dma_start_transpose(
                        a2tb[mt][:, NGB * 128:(NGB + NGC) * 128],
                        a2dc.ap()[:, mt * 128:(mt + 1) * 128])

                ot = [ow.tile([128, N], F32, tag=f"ot{mt}", name=f"ot{mt}")
                      for mt in range(3)]
                DCH = ((0, 16, 28, 448), (16, 12, 21, 336))
                for ci, (r0, nr, ngr, w) in enumerate(DCH):
                    csl = slice(0, 448) if ci == 0 else slice(448, 784)
                    half = a2ta if ci == 0 else a2tb
                    pp = [pe.tile([128, w], F32, tag=f"pp{ci}{mt}",
                                  name=f"pp{ci}{mt}") for mt in range(3)]
                    prev_e = None
                    for g in range(8):
                        po = pd.tile([128, w], F32, tag="po", name="po")
                        dgt = [dgp.tile([128, 128], BF16, tag="dg", name="dg")
                               for _ in range(9)]
                        for t in range(9):
                            nc.vector.tensor_scalar_mul(
                                dgt[t][:], ident_t[:], wtap_t[g][:, t:t + 1])
                        vv = vpad[g][:].rearrange("p (r c) -> p r c", c=30)
                        for t in range(9):
                            dy, dx = t // 3, t % 3
                            srcv = vv[:, r0 + dy:r0 + dy + nr, dx:dx + 28]
                            nc.tensor.matmul(po[:], lhsT=dgt[t][:], rhs=srcv,
                                             start=(t == 0), stop=False)
                        for mt in range(7):
                            cols = half[mt][:].rearrange(
                                "p (a G i) -> p a G i", G=8, i=16)[:, :, g, :]
                            nc.tensor.matmul(
                                po[:],
                                lhsT=vt[mt][:, g * 128:(g + 1) * 128],
                                rhs=cols, start=False, stop=(mt == 6))
                        nc.scalar.activation(osum[g][:, csl], po[:],
                                             AF.Identity, bias=bdw_t[g])
                        if prev_e is not None:
                            for mt in range(3):
                                nc.tensor.matmul(
                                    pp[mt][:],
                                    lhsT=wp_t[prev_e][:,
                                                      mt * 128:(mt + 1) * 128],
                                    rhs=osum[prev_e][:, csl],
                                    start=(prev_e == 0), stop=False)
                        prev_e = g
                    for mt in range(3):
                        nc.tensor.matmul(
                            pp[mt][:],
                            lhsT=wp_t[7][:, mt * 128:(mt + 1) * 128],
                            rhs=osum[7][:, csl],
                            start=False, stop=True)
                    for mt in range(3):
                        nc.scalar.activation(ot[mt][:, csl], pp[mt][:],
                                             AF.Identity, bias=bp_t[mt])
                        nc.scalar.dma_start(
                            out.ap()[mt * 128:(mt + 1) * 128, csl],
                            ot[mt][:, csl])

            a2tap_cm.__exit__(None, None, None)

    nc.compile()
    return nc


def _prep_common(inputs):
    f32 = np.float32
    scale = np.float32(KD ** -0.5)
    q_s, q_b = inputs["q_s"], inputs["q_b"]
    k_s, k_b = inputs["k_s"], inputs["k_b"]
    v_s, v_b = inputs["v_s"], inputs["v_b"]
    p_s, p_b = inputs["p_s"], inputs["p_b"]

    Wq = np.asarray(inputs["Wq"], f32) * np.asarray(q_s, f32)[:, None] * scale
    bqv = (np.asarray(q_s, f32) * np.asarray(inputs["bq"], f32)
           + np.asarray(q_b, f32)) * scale
    Wk = np.asarray(inputs["Wk"], f32) * np.asarray(k_s, f32)[:, None]
    bkv = np.asarray(k_s, f32) * np.asarray(inputs["bk"], f32) + np.asarray(k_b, f32)
    Wv = np.asarray(inputs["Wv"], f32) * np.asarray(v_s, f32)[:, None]
    bvv = np.asarray(v_s, f32) * np.asarray(inputs["bv"], f32) + np.asarray(v_b, f32)
    Wp = np.asarray(inputs["Wp"], f32) * np.asarray(p_s, f32)[:, None]
    bpv = np.asarray(p_s, f32) * np.asarray(inputs["bp"], f32) + np.asarray(p_b, f32)

    Wth1 = np.asarray(inputs["Wth1"], f32)
    bth1 = np.asarray(inputs["bth1"], f32)
    Wth2 = np.asarray(inputs["Wth2"], f32)
    bth2 = np.asarray(inputs["bth2"], f32)

    # talking-head-1 folded bias table, rows ordered (group, g, i); bth1 is
    # applied separately as the ACT exp bias
    ab1 = Wth1 @ np.asarray(inputs["attention_biases"], f32)      # [8, 784]
    idx = np.asarray(inputs["bias_idxs"])                          # [784, 784]
    ab_full = ab1[:, idx]                                          # [8,784,784]
    abt = np.ascontiguousarray(
        ab_full.reshape(8, NG, 16, N).transpose(1, 0, 2, 3)
    ).reshape(NG * 128, N).astype(F8)

    # depthwise weights folded with BN
    wvl = np.asarray(inputs["Wvl"], f32)[:, 0, :, :].reshape(DH, 9)
    vl_s = np.asarray(inputs["vl_s"], f32)
    wtap = wvl * vl_s[:, None]
    bdw = (np.asarray(inputs["bvl"], f32) * vl_s
           + np.asarray(inputs["vl_b"], f32))

    def ktile_pack(wT, nk):
        # [nk*128, C] -> [128, nk*C] with k-tile-major free dim
        C = wT.shape[1]
        return np.ascontiguousarray(
            wT.reshape(nk, 128, C).transpose(1, 0, 2).reshape(128, nk * C))

    sqv = np.repeat(Wth1.T, KD, axis=0).astype(f32)                # [256, 8]
    vecs = np.zeros((128, 113), f32)
    vecs[:, 0:2] = bqv.reshape(2, 128).T
    vecs[:, 2:4] = bkv.reshape(2, 128).T
    vecs[:, 4:12] = bvv.reshape(8, 128).T
    s2 = Wth2.sum(axis=1) + N * bth2                   # [8] per out-head
    bdw2 = bdw + bvv * np.repeat(s2, D)
    vecs[:, 12:20] = bdw2.reshape(8, 128).T
    vecs[:, 20:23] = bpv.reshape(3, 128).T
    vecs[:, 23] = np.repeat(bth1, 16)
    vecs[:, 24] = np.repeat(bth2, 16)
    vecs[:, 25:33] = sqv[0:128]
    vecs[:, 33:41] = sqv[128:256]
    for g in range(8):
        vecs[:, 41 + g * 9:50 + g * 9] = wtap[g * 128:(g + 1) * 128]

    common = {
        "wq3": ktile_pack(np.ascontiguousarray(Wq.T), 3).astype(BF),
        "wk3": ktile_pack(np.ascontiguousarray(Wk.T), 3).astype(BF),
        "wv3": ktile_pack(np.ascontiguousarray(Wv.T), 3).astype(BF),
        "wp8": ktile_pack(np.ascontiguousarray(Wp.T), 8).astype(BF),
        "vecs": vecs,
        "w2bd": np.kron(Wth2.T, np.eye(16, dtype=f32)).astype(f32),
        "ident": np.eye(128, dtype=f32).astype(BF),
        "identf8": np.eye(128, dtype=f32).astype(F8),
        "abt": abt,
    }
    return common


def kernel(**inputs):
    global LAST_RESULTS
    if "nc" not in _CACHE:
        _CACHE["nc"] = _build_program()
    nc = _CACHE["nc"]

    common = _prep_common(inputs)
    x = np.asarray(inputs["x"], np.float32)          # [8, 384, 28, 28]
    in_maps = []
    for c in range(B):
        m = dict(common)
        xc = x[c].reshape(3, 128, N).transpose(1, 0, 2).reshape(128, 3 * N)
        m["x_c"] = np.ascontiguousarray(xc).astype(BF)
        in_maps.append(m)

    import os
    trace = bool(int(os.environ.get("KERNEL_TRACE", "0")))
    res = run_bass_kernel_spmd(nc, in_maps, core_ids=list(range(B)),
                               trace=trace)
    LAST_RESULTS = res
    out = np.stack([res.results[c]["out"].reshape(DIM, RES, RES)
                    for c in range(B)])
    return out.astype(np.float32)
